# revision 1
# baseline (speedup 1.0000x reference)
"""AttnSenseNet Trainium2 kernel.

Strategy (8 NeuronCores):
  - Batch-parallel attention front-end: each core handles 8 of the 64 batch
    rows.  Embedding rows are fetched with dma_gather (int16 indices) from a
    bf16 table laid out as 4 quarter-blocks of 25001 rows, each ending in a
    zero row; every slot carries a valid index (its local row in the matching
    quarter, or that quarter's zero row), so the 4 per-quarter gather outputs
    simply sum to the gathered embeddings.
  - Word/sense attention computed with DVE (d-contractions as mult+reduce
    along the free dim) and PE (l/n-contractions as matmuls over the partition
    dim).  Cross-partition broadcasts go through PE (all-constant or
    stride-0-free-dim stationary operands); compute engines cannot read
    stride-0 partition APs.
  - Vocab-parallel classifier: hidden vectors all-gathered (tiny), each core
    computes logits for its 6250-column shard of W_lin^T (bf16), log-softmax
    stats combined with a second tiny all-gather.  b_lin enters via a
    partition-broadcast DMA load fused into the PSUM->SBUF add.
  - Host-side input marshalling only: W_lin transpose + bf16 cast, index
    remap/permutation, table quarter layout + pad-row zeroing, W_attn/3 fold.

Output: [64, 50000] float32 log-softmax, assembled by concatenating the 8
per-core [64, 6250] shards along axis 1.
"""

import os
import sys

import numpy as np

sys.path.insert(0, "/opt/trn_rl_repo")

LAST_EXEC_NS = None
LAST_RESULTS = None

N_CORES = 8
B = 64
BSH = 8          # batch rows per core
NH = 2           # halves (gather granularity): 4 batch rows each
BH = BSH // NH
L = 512
S = 3
D = 128
C = 4            # l-chunks of 128
P = 128
VOCAB = 100000
QROWS = VOCAB // 4           # 25000 rows per table quarter
QSTRIDE = QROWS + 1          # +1 zero row per quarter
OV = 50000
VSH = OV // N_CORES          # 6250 vocab columns per core
NCHUNK = 512                 # logits matmul moving-dim chunk
NIDX_H = BH * C * S * P      # 6144 gathered rows per half
MASK_NEG = np.float32(-1e30)


def _chunks():
    out = []
    off = 0
    while off < VSH:
        n = min(NCHUNK, VSH - off)
        out.append((off, n))
        off += n
    return out


def build_nc():
    import concourse.bass as bass
    import concourse.bacc as bacc
    import concourse.tile as tile
    from concourse import mybir

    f32 = mybir.dt.float32
    bf16 = mybir.dt.bfloat16
    i16 = mybir.dt.int16
    AF = mybir.ActivationFunctionType
    AL = mybir.AluOpType
    AX = mybir.AxisListType

    nc = bacc.Bacc("TRN2", target_bir_lowering=False, debug=False,
                   num_devices=N_CORES)

    table = nc.dram_tensor("table", [4 * QSTRIDE, D], bf16,
                           kind="ExternalInput").ap()
    idx_d = {}
    for h in range(NH):
        for q in range(4):
            nm = f"idx{h}{q}"
            idx_d[(h, q)] = nc.dram_tensor(
                nm, [P, NIDX_H // 16], i16, kind="ExternalInput").ap()
    maskb = nc.dram_tensor("maskb", [P, BSH * C], f32, kind="ExternalInput").ap()
    w4 = nc.dram_tensor("w4", [1, C * D], bf16, kind="ExternalInput").ap()
    lwin = nc.dram_tensor("lw", [1, BSH], f32, kind="ExternalInput").ap()
    wlint = nc.dram_tensor("wlint", [D, VSH], bf16, kind="ExternalInput").ap()
    blin = nc.dram_tensor("blin", [1, VSH], f32, kind="ExternalInput").ap()
    ident = nc.dram_tensor("ident", [P, P], f32, kind="ExternalInput").ap()
    out = nc.dram_tensor("out", [B, VSH], f32, kind="ExternalOutput").ap()

    def bcast_dram(ap, nparts, n):
        # stride-0 partition-broadcast read of a [1, n] DRAM row (DMA only)
        return bass.AP(tensor=ap.tensor, offset=ap.offset,
                       ap=[[0, nparts], [1, n]])

    from contextlib import ExitStack

    with tile.TileContext(nc) as tc, ExitStack() as ctx:
        const = ctx.enter_context(tc.tile_pool(name="const", bufs=1))
        big = ctx.enter_context(tc.tile_pool(name="big", bufs=1))
        gath = ctx.enter_context(tc.tile_pool(name="gath", bufs=1))
        work = ctx.enter_context(tc.tile_pool(name="work", bufs=3))
        simp = ctx.enter_context(tc.tile_pool(name="simp", bufs=2))
        escp = ctx.enter_context(tc.tile_pool(name="escp", bufs=2))
        pacc = ctx.enter_context(tc.tile_pool(name="pacc", bufs=2, space="PSUM"))
        pws = ctx.enter_context(tc.tile_pool(name="pws", bufs=1, space="PSUM"))
        pctx = ctx.enter_context(tc.tile_pool(name="pctx", bufs=2, space="PSUM"))
        ptp = ctx.enter_context(tc.tile_pool(name="ptp", bufs=1, space="PSUM"))
        plog = ctx.enter_context(tc.tile_pool(name="plog", bufs=2, space="PSUM"))
        dram = ctx.enter_context(tc.tile_pool(name="dram", bufs=1, space="DRAM"))

        # ---- constant / input loads (HWDGE) ----
        idx_sb = {}
        for h in range(NH):
            for q in range(4):
                t = const.tile([P, NIDX_H // 16], i16, tag=f"idx{h}{q}", name=f"idxsb{h}{q}")
                nc.sync.dma_start(out=t[:], in_=idx_d[(h, q)])
                idx_sb[(h, q)] = t
        maskb_sb = const.tile([P, BSH * C], f32)
        nc.sync.dma_start(out=maskb_sb[:], in_=maskb)
        w4_sb = const.tile([P, C * D], bf16)          # W_attn/3 tiled, all parts
        nc.sync.dma_start(out=w4_sb[:], in_=bcast_dram(w4, P, C * D))
        lw_sb = const.tile([P, BSH], f32)             # length_weights, all parts
        nc.sync.dma_start(out=lw_sb[:], in_=bcast_dram(lwin, P, BSH))
        ident_sb = const.tile([P, P], f32)
        nc.sync.dma_start(out=ident_sb[:], in_=ident)
        w_sb = const.tile([D, VSH], bf16)
        nc.sync.dma_start(out=w_sb[:], in_=wlint)
        b_bc = const.tile([B, VSH], f32)              # b_lin on 64 partitions
        nc.sync.dma_start(out=b_bc[:], in_=bcast_dram(blin, B, VSH))
        threes = const.tile([P, P], bf16)             # all 3.0 (partition sums)
        nc.vector.memset(threes[:], 3.0)
        ones8 = const.tile([BSH, 1], f32)
        nc.vector.memset(ones8[:], 1.0)

        hidT = big.tile([P, BSH], f32)     # hidden^T columns (d on partitions)

        emb_bs = {}
        ctxbc_bs = {}
        for h in range(NH):
            # ---- gather: 4 quarter-gathers, then sum (zero rows elsewhere)
            emb_h = big.tile([P, NIDX_H // P, P], bf16, tag=f"embh{h}")
            dq = []
            for q in range(1, 4):
                dq.append(gath.tile([P, NIDX_H // P, P], bf16, tag=f"q{q}", name=f"dq{q}"))
            for q in range(4):
                dst = emb_h if q == 0 else dq[q - 1]
                nc.gpsimd.dma_gather(
                    out_ap=dst[:], in_ap=table[q * QSTRIDE:(q + 1) * QSTRIDE, :],
                    idxs_ap=idx_sb[(h, q)][:],
                    num_idxs=NIDX_H, num_idxs_reg=NIDX_H, elem_size=D,
                    single_packet=False)
            ehf = emb_h[:].rearrange("p a d -> p (a d)")
            d1f = dq[0][:].rearrange("p a d -> p (a d)")
            d2f = dq[1][:].rearrange("p a d -> p (a d)")
            d3f = dq[2][:].rearrange("p a d -> p (a d)")
            # combine quarter-gathers (zero rows make plain sums correct)
            nc.vector.tensor_tensor(out=ehf, in0=ehf, in1=d1f, op=AL.add)
            nc.vector.tensor_tensor(out=d2f, in0=d2f, in1=d3f, op=AL.add)
            nc.vector.tensor_tensor(out=ehf, in0=ehf, in1=d2f, op=AL.add)

            for bl in range(BH):
                b = h * BH + bl
                # emb_b[p, (c,s)*128+d], row (b, l=c*128+p, sense s)
                emb_b = emb_h[:].rearrange("p a d -> p (a d)")[
                    :, bl * C * S * D:(bl + 1) * C * S * D]
                emb_bs[b] = emb_b

                # sense-sum (3*mean): embsum_b[p, c*128+d] = sum_s emb_b
                eb4 = emb_b.rearrange("p (c s d) -> p c s d", s=S, d=D)
                embsum_b = big.tile([P, C * D], bf16, tag=f"esum{b}")
                es4 = embsum_b[:].rearrange("p (c d) -> p c d", d=D)
                nc.vector.tensor_tensor(out=es4, in0=eb4[:, :, 0, :],
                                        in1=eb4[:, :, 1, :], op=AL.add)
                nc.vector.tensor_tensor(out=es4, in0=es4,
                                        in1=eb4[:, :, 2, :], op=AL.add)

                # word importance: wimp_b[p, c] = sum_d embsum_b * (W_attn/3)
                wtmp = work.tile([P, C * D], bf16, tag="wtmp")
                nc.vector.tensor_tensor(out=wtmp[:], in0=embsum_b[:],
                                        in1=w4_sb[:], op=AL.mult)
                wimp_b = work.tile([P, C], f32, tag="wimp")
                nc.vector.reduce_sum(
                    out=wimp_b[:],
                    in_=wtmp[:].rearrange("p (c d) -> p c d", d=D),
                    axis=AX.X)
                # mask, exp (word softmax numerator; |wimp| << 1, no max-sub)
                nc.vector.tensor_tensor(out=wimp_b[:], in0=wimp_b[:],
                                        in1=maskb_sb[:, b * C:(b + 1) * C],
                                        op=AL.add)
                e_b = work.tile([P, C], bf16, tag=f"e{b}")
                nc.scalar.activation(out=e_b[:], in_=wimp_b[:], func=AF.Exp)

                # 3*sum_l e, replicated on every partition (all-threes matmul)
                ws_ps = pws.tile([P, C], f32, tag="ws")
                nc.tensor.matmul(out=ws_ps[:], lhsT=threes[:], rhs=e_b[:],
                                 start=True, stop=True)
                s3_b = work.tile([P, 1], f32, tag="s3w")
                nc.vector.reduce_sum(out=s3_b[:], in_=ws_ps[:], axis=AX.X)
                r_b = work.tile([P, 1], f32, tag="rb")
                nc.vector.reciprocal(out=r_b[:], in_=s3_b[:])

                # context, replicated on all partitions: PE outer products
                ctx_ps = pctx.tile([P, D], f32, tag="ctxps")
                for c in range(C):
                    nc.tensor.matmul(
                        out=ctx_ps[:],
                        lhsT=e_b[:, c:c + 1].to_broadcast([P, P]),
                        rhs=embsum_b[:, c * D:(c + 1) * D],
                        start=(c == 0), stop=(c == C - 1))
                ctxbc_b = big.tile([P, D], bf16, tag=f"ctx{b}")
                nc.scalar.activation(out=ctxbc_b[:], in_=ctx_ps[:],
                                     func=AF.Copy, scale=r_b[:])
                ctxbc_bs[b] = ctxbc_b

        for b in range(BSH):
            emb_b = emb_bs[b]
            ctxbc_b = ctxbc_bs[b]
            # sim_b[p, (c,s)] = sum_d emb_b * context_b
            stmp = simp.tile([P, C * S * D], bf16, tag="stmp")
            _cap = ctxbc_b[:]
            ctx_rep = bass.AP(tensor=_cap.tensor, offset=_cap.offset,
                              ap=[_cap.ap[0], [0, C * S], [1, D]])
            nc.vector.tensor_tensor(
                out=stmp[:].rearrange("p (j d) -> p j d", d=D),
                in0=emb_b.rearrange("p (j d) -> p j d", d=D),
                in1=ctx_rep, op=AL.mult)
            sim_b = work.tile([P, C * S], f32, tag="sim")
            nc.vector.reduce_sum(
                out=sim_b[:],
                in_=stmp[:].rearrange("p (j d) -> p j d", d=D),
                axis=AX.X)
            # sense softmax (groups of 3; |sim| << 1, no max-sub) and
            # final attention weights w = lw * e3 / sum3
            e3_b = work.tile([P, C * S], f32, tag="e3")
            nc.scalar.activation(out=e3_b[:], in_=sim_b[:], func=AF.Exp)
            e3v = e3_b[:].rearrange("p (c s) -> p c s", s=S)
            s3s = work.tile([P, C], f32, tag="s3s")
            nc.vector.tensor_tensor(out=s3s[:], in0=e3v[:, :, 0],
                                    in1=e3v[:, :, 1], op=AL.add)
            nc.vector.tensor_tensor(out=s3s[:], in0=s3s[:],
                                    in1=e3v[:, :, 2], op=AL.add)
            r3s = work.tile([P, C], f32, tag="r3s")
            nc.vector.reciprocal(out=r3s[:], in_=s3s[:])
            nc.vector.tensor_scalar_mul(out=r3s[:], in0=r3s[:],
                                        scalar1=lw_sb[:, b:b + 1])
            w_b = work.tile([P, C * S], bf16, tag="wb")
            wbv = w_b[:].rearrange("p (c s) -> p c s", s=S)
            for s in range(S):
                nc.vector.tensor_tensor(out=wbv[:, :, s], in0=e3v[:, :, s],
                                        in1=r3s[:], op=AL.mult)
            # hidden^T column: sum_n w_n * emb_n (PE over partitions, 12 blocks)
            hid_ps = pacc.tile([P, 1], f32, tag="acc")
            for j in range(C * S):
                nc.tensor.matmul(out=hid_ps[:],
                                 lhsT=emb_b[:, j * D:(j + 1) * D],
                                 rhs=w_b[:, j:j + 1],
                                 start=(j == 0), stop=(j == C * S - 1))
            nc.vector.tensor_copy(out=hidT[:, b:b + 1], in_=hid_ps[:])

        # ---- all-gather hidden: [8,128] local -> [64,128] global ----
        ht_ps = ptp.tile([BSH, P], f32, tag="tp")
        nc.tensor.transpose(out=ht_ps[:], in_=hidT[:], identity=ident_sb[:])
        hid8 = big.tile([BSH, P], f32)
        nc.scalar.copy(out=hid8[:], in_=ht_ps[:])
        hin = dram.tile([BSH, P], f32)
        hout = dram.tile([B, P], f32)
        nc.sync.dma_start(out=hin[:], in_=hid8[:])
        nc.gpsimd.collective_compute(
            "AllGather",
            mybir.AluOpType.bypass,
            ins=[hin[:].opt()],
            outs=[hout[:].opt()],
            replica_groups=[list(range(N_CORES))],
        )
        hid64 = big.tile([B, P], f32)
        nc.sync.dma_start(out=hid64[:], in_=hout[:])
        h64_ps = ptp.tile([P, B], f32, tag="tp")
        nc.tensor.transpose(out=h64_ps[:], in_=hid64[:],
                            identity=ident_sb[:B, :B])
        hidT64 = big.tile([P, B], bf16)
        nc.scalar.copy(out=hidT64[:], in_=h64_ps[:])

        # ---- logits shard + exp-sum stats ----
        y_all = big.tile([B, VSH], f32)
        acc = big.tile([B, 16], f32)
        chs = _chunks()
        for ci, (off, n) in enumerate(chs):
            lp = plog.tile([B, NCHUNK], f32, tag="log")
            nc.tensor.matmul(out=lp[:, :n], lhsT=hidT64[:],
                             rhs=w_sb[:, off:off + n], start=True, stop=True)
            nc.vector.tensor_tensor(out=y_all[:, off:off + n], in0=lp[:, :n],
                                    in1=b_bc[:, off:off + n], op=AL.add)
            esc = escp.tile([B, NCHUNK], f32, tag="esc")
            nc.scalar.activation(out=esc[:, :n], in_=y_all[:, off:off + n],
                                 func=AF.Exp, accum_out=acc[:, ci:ci + 1])
        sloc = big.tile([B, 1], f32)
        nc.vector.reduce_sum(out=sloc[:], in_=acc[:, :len(chs)], axis=AX.X)

        # ---- all-gather per-core exp-sums, combine, normalize ----
        sin = dram.tile([B, 1], f32)
        sout = dram.tile([N_CORES, B], f32)
        nc.sync.dma_start(out=sin[:], in_=sloc[:])
        nc.gpsimd.collective_compute(
            "AllGather",
            mybir.AluOpType.bypass,
            ins=[sin[:].opt()],
            outs=[sout[:].opt()],
            replica_groups=[list(range(N_CORES))],
        )
        s8 = big.tile([N_CORES, B], f32)
        nc.sync.dma_start(out=s8[:], in_=sout[:])
        st_ps = ptp.tile([B, 1], f32, tag="tp")
        nc.tensor.matmul(out=st_ps[:], lhsT=s8[:], rhs=ones8[:],
                         start=True, stop=True)
        logz = big.tile([B, 1], f32)
        nc.scalar.activation(out=logz[:], in_=st_ps[:], func=AF.Ln)
        nc.vector.tensor_scalar_sub(out=y_all[:], in0=y_all[:],
                                    scalar1=logz[:])
        nc.sync.dma_start(out=out, in_=y_all[:])

    nc.compile()
    return nc


def _wrap16(v):
    """dma_gather index layout: position i -> (i % 16, i // 16), replicated
    onto 128 partitions (8 Q7 cores x 16)."""
    w = v.reshape(-1, 16).T
    return np.ascontiguousarray(np.tile(w, (8, 1)))


def prepare_in_maps(inputs):
    import ml_dtypes

    bf16 = ml_dtypes.bfloat16
    inp = np.asarray(inputs["inputs"]).astype(np.int64)           # [64, 1536]
    lw = np.asarray(inputs["length_weights"]).astype(np.float32).reshape(B)
    mask = np.asarray(inputs["word_attn_mask"]).astype(bool)      # [64, 512]
    emb = np.asarray(inputs["embedding"]).astype(np.float32).copy()
    emb[0, :] = 0.0                                               # padding row
    w_attn = np.asarray(inputs["W_attn"]).astype(np.float32).reshape(D)
    # b_attn is softmax-invariant (constant shift before word softmax): ignored
    w_lin = np.asarray(inputs["W_lin"]).astype(np.float32)        # [50000, 128]
    b_lin = np.asarray(inputs["b_lin"]).astype(np.float32).reshape(OV)

    # table: 4 quarter-blocks of QSTRIDE rows, each ending in a zero row
    tbl = np.zeros((4 * QSTRIDE, D), dtype=bf16)
    for q in range(4):
        tbl[q * QSTRIDE:q * QSTRIDE + QROWS] = emb[
            q * QROWS:(q + 1) * QROWS].astype(bf16)

    wt = np.ascontiguousarray(w_lin.T).astype(bf16)               # [128, 50000]
    w4 = np.tile((w_attn / 3.0), C)[None, :].astype(bf16)         # [1, 512]
    ident = np.eye(P, dtype=np.float32)

    # token order within a half: i = (b_loc*12 + c*3 + s)*128 + p
    # maps token (b = h*4+b_loc, l = c*128+p, sense s)
    idx6 = inp.reshape(N_CORES, NH, BH, C, P, S)       # (core,h,bl,c,p,s)
    pos = idx6.transpose(0, 1, 2, 3, 5, 4).reshape(N_CORES, NH, BH * C * S, P)
    pos = pos.transpose(0, 1, 3, 2)                    # (core, h, p, f)
    # flat order i = f*128 + p:
    flat = pos.transpose(0, 1, 3, 2).reshape(N_CORES, NH, NIDX_H)

    quarter = flat // QROWS                            # 0..3
    local = (flat % QROWS).astype(np.int64)
    idx_dev = {}
    for h in range(NH):
        for q in range(4):
            v = np.where(quarter[:, h] == q, local[:, h], QROWS).astype(np.int16)
            idx_dev[(h, q)] = np.stack(
                [_wrap16(v[c]) for c in range(N_CORES)])

    mb6 = np.where(mask, MASK_NEG, np.float32(0.0)).astype(
        np.float32).reshape(N_CORES, BSH, C, P)
    maskb_dev = np.ascontiguousarray(
        mb6.transpose(0, 3, 1, 2).reshape(N_CORES, P, BSH * C))
    lw_dev = lw.reshape(N_CORES, 1, BSH)

    in_maps = []
    for c in range(N_CORES):
        m = {
            "table": tbl,
            "maskb": maskb_dev[c],
            "w4": w4,
            "lw": np.ascontiguousarray(lw_dev[c]),
            "wlint": np.ascontiguousarray(wt[:, c * VSH:(c + 1) * VSH]),
            "blin": np.ascontiguousarray(b_lin[c * VSH:(c + 1) * VSH][None, :]),
            "ident": ident,
        }
        for h in range(NH):
            for q in range(4):
                m[f"idx{h}{q}"] = np.ascontiguousarray(idx_dev[(h, q)][c])
        in_maps.append(m)
    return in_maps


def _install_ntff_hook():
    """Provide antenv.axon_hooks (NTFF profiling glue) if the image lacks it.

    bass_utils hard-imports it on the trace=True path; this container's
    antenv package does not ship the module even though the axon .so
    supports profiling.  No-op if the real module exists or anything fails.
    """
    try:
        import importlib.util
        if "antenv.axon_hooks" in sys.modules:
            return
        try:
            if importlib.util.find_spec("antenv.axon_hooks") is not None:
                return
        except ModuleNotFoundError:
            pass
        import contextlib
        import ctypes
        import types

        so_path = "/opt/axon/libaxon_pjrt.so"
        if not os.path.exists(so_path):
            return
        lib = ctypes.CDLL(so_path)
        if not hasattr(lib, "axon_start_nrt_profile"):
            return
        lib.axon_start_nrt_profile.argtypes = [
            ctypes.POINTER(ctypes.c_int64), ctypes.c_size_t]
        lib.axon_start_nrt_profile.restype = ctypes.c_int64
        lib.axon_stop_nrt_profile.argtypes = [ctypes.c_char_p]
        lib.axon_stop_nrt_profile.restype = ctypes.c_int64

        @contextlib.contextmanager
        def _hook(output_dir, device_ids):
            import jax
            jax.devices()
            if device_ids:
                ids = (ctypes.c_int64 * len(device_ids))(*device_ids)
                rc = lib.axon_start_nrt_profile(ids, len(device_ids))
            else:
                rc = lib.axon_start_nrt_profile(None, 0)
            if rc != 0:
                raise RuntimeError(f"axon_start_nrt_profile rc={rc}")
            try:
                yield
            finally:
                n = lib.axon_stop_nrt_profile(str(output_dir).encode())
                print(f"profile: {n} file(s) written to {output_dir}",
                      file=sys.stderr)

        mod = types.ModuleType("antenv.axon_hooks")
        mod.get_axon_ntff_profile_hook = lambda: _hook
        mod.set_axon_ntff_profile_hook = lambda h: None
        sys.modules["antenv.axon_hooks"] = mod
        try:
            import antenv
            antenv.axon_hooks = mod
        except Exception:
            pass
    except Exception:
        pass


def kernel(**inputs):
    global LAST_EXEC_NS, LAST_RESULTS
    _install_ntff_hook()
    from concourse import bass_utils

    nc = build_nc()
    in_maps = prepare_in_maps(inputs)
    res = bass_utils.run_bass_kernel_spmd(
        nc, in_maps, core_ids=list(range(N_CORES)))
    LAST_EXEC_NS = res.exec_time_ns
    LAST_RESULTS = res
    return np.concatenate(
        [res.results[c]["out"] for c in range(N_CORES)], axis=1
    ).astype(np.float32)



# revision 2
# speedup vs baseline: 3.1403x; 3.1403x over previous
"""AttnSenseNet Trainium2 kernel.

Strategy (8 NeuronCores):
  - Batch-parallel attention front-end: each core handles 8 of the 64 batch
    rows.  Embedding rows are fetched with dma_gather (int16 indices) from a
    per-core COMPACTED bf16 table: the host dedups the <=12288 distinct rows
    this core's tokens touch (12288 < 2^15, so one int16-indexed gather per
    half covers everything — no quarter split, no zero-row redundancy).
  - Word/sense attention computed with DVE (d-contractions as mult+reduce
    along the free dim) and PE (l/n-contractions as matmuls over the partition
    dim).  Cross-partition broadcasts go through PE (all-constant or
    stride-0-free-dim stationary operands); compute engines cannot read
    stride-0 partition APs.
  - Vocab-parallel classifier: hidden vectors all-gathered (tiny), each core
    computes logits for its 6250-column shard of W_lin^T (bf16), log-softmax
    stats combined with a second tiny all-gather.  b_lin enters via a
    partition-broadcast DMA load fused into the PSUM->SBUF add.
  - Host-side input marshalling only: W_lin transpose + bf16 cast, per-core
    row dedup + index remap/permutation, W_attn/3 fold.

Output: [64, 50000] float32 log-softmax, assembled by concatenating the 8
per-core [64, 6250] shards along axis 1.
"""

import os
import sys

import numpy as np

sys.path.insert(0, "/opt/trn_rl_repo")

LAST_EXEC_NS = None
LAST_RESULTS = None

N_CORES = 8
B = 64
BSH = 8          # batch rows per core
NH = 2           # halves (gather granularity): 4 batch rows each
BH = BSH // NH
L = 512
S = 3
D = 128
C = 4            # l-chunks of 128
P = 128
VOCAB = 100000
TROWS = 12288                # compact per-core table rows (>= distinct rows)
OV = 50000
VSH = OV // N_CORES          # 6250 vocab columns per core
NCHUNK = 512                 # logits matmul moving-dim chunk
NIDX_H = BH * C * S * P      # 6144 gathered rows per half
MASK_NEG = np.float32(-1e30)


def _chunks():
    out = []
    off = 0
    while off < VSH:
        n = min(NCHUNK, VSH - off)
        out.append((off, n))
        off += n
    return out


def build_nc():
    import concourse.bass as bass
    import concourse.bacc as bacc
    import concourse.tile as tile
    from concourse import mybir

    f32 = mybir.dt.float32
    bf16 = mybir.dt.bfloat16
    i16 = mybir.dt.int16
    AF = mybir.ActivationFunctionType
    AL = mybir.AluOpType
    AX = mybir.AxisListType

    nc = bacc.Bacc("TRN2", target_bir_lowering=False, debug=False,
                   num_devices=N_CORES)

    table = nc.dram_tensor("table", [TROWS, D], bf16,
                           kind="ExternalInput").ap()
    idx_d = {}
    for h in range(NH):
        idx_d[h] = nc.dram_tensor(
            f"idx{h}", [P, NIDX_H // 16], i16, kind="ExternalInput").ap()
    maskb = nc.dram_tensor("maskb", [P, BSH * C], f32, kind="ExternalInput").ap()
    w4 = nc.dram_tensor("w4", [1, C * D], bf16, kind="ExternalInput").ap()
    lwin = nc.dram_tensor("lw", [1, BSH], f32, kind="ExternalInput").ap()
    wlint = nc.dram_tensor("wlint", [D, VSH], bf16, kind="ExternalInput").ap()
    blin = nc.dram_tensor("blin", [1, VSH], f32, kind="ExternalInput").ap()
    ident = nc.dram_tensor("ident", [P, P], f32, kind="ExternalInput").ap()
    out = nc.dram_tensor("out", [B, VSH], f32, kind="ExternalOutput").ap()

    def bcast_dram(ap, nparts, n):
        # stride-0 partition-broadcast read of a [1, n] DRAM row (DMA only)
        return bass.AP(tensor=ap.tensor, offset=ap.offset,
                       ap=[[0, nparts], [1, n]])

    from contextlib import ExitStack

    with tile.TileContext(nc) as tc, ExitStack() as ctx:
        const = ctx.enter_context(tc.tile_pool(name="const", bufs=1))
        big = ctx.enter_context(tc.tile_pool(name="big", bufs=1))
        work = ctx.enter_context(tc.tile_pool(name="work", bufs=3))
        simp = ctx.enter_context(tc.tile_pool(name="simp", bufs=2))
        escp = ctx.enter_context(tc.tile_pool(name="escp", bufs=2))
        pacc = ctx.enter_context(tc.tile_pool(name="pacc", bufs=2, space="PSUM"))
        pws = ctx.enter_context(tc.tile_pool(name="pws", bufs=1, space="PSUM"))
        pctx = ctx.enter_context(tc.tile_pool(name="pctx", bufs=2, space="PSUM"))
        ptp = ctx.enter_context(tc.tile_pool(name="ptp", bufs=1, space="PSUM"))
        plog = ctx.enter_context(tc.tile_pool(name="plog", bufs=2, space="PSUM"))
        dram = ctx.enter_context(tc.tile_pool(name="dram", bufs=1, space="DRAM"))

        # ---- constant / input loads (HWDGE) ----
        idx_sb = {}
        for h in range(NH):
            t = const.tile([P, NIDX_H // 16], i16, tag=f"idx{h}", name=f"idxsb{h}")
            nc.sync.dma_start(out=t[:], in_=idx_d[h])
            idx_sb[h] = t
        maskb_sb = const.tile([P, BSH * C], f32)
        nc.sync.dma_start(out=maskb_sb[:], in_=maskb)
        w4_sb = const.tile([P, C * D], bf16)          # W_attn/3 tiled, all parts
        nc.sync.dma_start(out=w4_sb[:], in_=bcast_dram(w4, P, C * D))
        lw_sb = const.tile([P, BSH], f32)             # length_weights, all parts
        nc.sync.dma_start(out=lw_sb[:], in_=bcast_dram(lwin, P, BSH))
        ident_sb = const.tile([P, P], f32)
        nc.sync.dma_start(out=ident_sb[:], in_=ident)
        w_sb = const.tile([D, VSH], bf16)
        nc.sync.dma_start(out=w_sb[:], in_=wlint)
        b_bc = const.tile([B, VSH], f32)              # b_lin on 64 partitions
        nc.sync.dma_start(out=b_bc[:], in_=bcast_dram(blin, B, VSH))
        threes = const.tile([P, P], bf16)             # all 3.0 (partition sums)
        nc.vector.memset(threes[:], 3.0)
        ones8 = const.tile([BSH, 1], f32)
        nc.vector.memset(ones8[:], 1.0)

        hidT = big.tile([P, BSH], f32)     # hidden^T columns (d on partitions)

        emb_bs = {}
        ctxbc_bs = {}
        for h in range(NH):
            # ---- single compact gather per half (rows deduped on host) ----
            emb_h = big.tile([P, NIDX_H // P, P], bf16, tag=f"embh{h}")
            nc.gpsimd.dma_gather(
                out_ap=emb_h[:], in_ap=table[0:TROWS, :],
                idxs_ap=idx_sb[h][:],
                num_idxs=NIDX_H, num_idxs_reg=NIDX_H, elem_size=D,
                single_packet=False)

            for bl in range(BH):
                b = h * BH + bl
                # emb_b[p, (c,s)*128+d], row (b, l=c*128+p, sense s)
                emb_b = emb_h[:].rearrange("p a d -> p (a d)")[
                    :, bl * C * S * D:(bl + 1) * C * S * D]
                emb_bs[b] = emb_b

                # sense-sum (3*mean): embsum_b[p, c*128+d] = sum_s emb_b
                eb4 = emb_b.rearrange("p (c s d) -> p c s d", s=S, d=D)
                embsum_b = big.tile([P, C * D], bf16, tag=f"esum{b}")
                es4 = embsum_b[:].rearrange("p (c d) -> p c d", d=D)
                nc.vector.tensor_tensor(out=es4, in0=eb4[:, :, 0, :],
                                        in1=eb4[:, :, 1, :], op=AL.add)
                nc.vector.tensor_tensor(out=es4, in0=es4,
                                        in1=eb4[:, :, 2, :], op=AL.add)

                # word importance: wimp_b[p, c] = sum_d embsum_b * (W_attn/3)
                wtmp = work.tile([P, C * D], bf16, tag="wtmp")
                nc.vector.tensor_tensor(out=wtmp[:], in0=embsum_b[:],
                                        in1=w4_sb[:], op=AL.mult)
                wimp_b = work.tile([P, C], f32, tag="wimp")
                nc.vector.reduce_sum(
                    out=wimp_b[:],
                    in_=wtmp[:].rearrange("p (c d) -> p c d", d=D),
                    axis=AX.X)
                # mask, exp (word softmax numerator; |wimp| << 1, no max-sub)
                nc.vector.tensor_tensor(out=wimp_b[:], in0=wimp_b[:],
                                        in1=maskb_sb[:, b * C:(b + 1) * C],
                                        op=AL.add)
                e_b = work.tile([P, C], bf16, tag=f"e{b}")
                nc.scalar.activation(out=e_b[:], in_=wimp_b[:], func=AF.Exp)

                # 3*sum_l e, replicated on every partition (all-threes matmul)
                ws_ps = pws.tile([P, C], f32, tag="ws")
                nc.tensor.matmul(out=ws_ps[:], lhsT=threes[:], rhs=e_b[:],
                                 start=True, stop=True)
                s3_b = work.tile([P, 1], f32, tag="s3w")
                nc.vector.reduce_sum(out=s3_b[:], in_=ws_ps[:], axis=AX.X)
                r_b = work.tile([P, 1], f32, tag="rb")
                nc.vector.reciprocal(out=r_b[:], in_=s3_b[:])

                # context, replicated on all partitions: PE outer products
                ctx_ps = pctx.tile([P, D], f32, tag="ctxps")
                for c in range(C):
                    nc.tensor.matmul(
                        out=ctx_ps[:],
                        lhsT=e_b[:, c:c + 1].to_broadcast([P, P]),
                        rhs=embsum_b[:, c * D:(c + 1) * D],
                        start=(c == 0), stop=(c == C - 1))
                ctxbc_b = big.tile([P, D], bf16, tag=f"ctx{b}")
                nc.scalar.activation(out=ctxbc_b[:], in_=ctx_ps[:],
                                     func=AF.Copy, scale=r_b[:])
                ctxbc_bs[b] = ctxbc_b

        for b in range(BSH):
            emb_b = emb_bs[b]
            ctxbc_b = ctxbc_bs[b]
            # sim_b[p, (c,s)] = sum_d emb_b * context_b
            stmp = simp.tile([P, C * S * D], bf16, tag="stmp")
            _cap = ctxbc_b[:]
            ctx_rep = bass.AP(tensor=_cap.tensor, offset=_cap.offset,
                              ap=[_cap.ap[0], [0, C * S], [1, D]])
            nc.vector.tensor_tensor(
                out=stmp[:].rearrange("p (j d) -> p j d", d=D),
                in0=emb_b.rearrange("p (j d) -> p j d", d=D),
                in1=ctx_rep, op=AL.mult)
            sim_b = work.tile([P, C * S], f32, tag="sim")
            nc.vector.reduce_sum(
                out=sim_b[:],
                in_=stmp[:].rearrange("p (j d) -> p j d", d=D),
                axis=AX.X)
            # sense softmax (groups of 3; |sim| << 1, no max-sub) and
            # final attention weights w = lw * e3 / sum3
            e3_b = work.tile([P, C * S], f32, tag="e3")
            nc.scalar.activation(out=e3_b[:], in_=sim_b[:], func=AF.Exp)
            e3v = e3_b[:].rearrange("p (c s) -> p c s", s=S)
            s3s = work.tile([P, C], f32, tag="s3s")
            nc.vector.tensor_tensor(out=s3s[:], in0=e3v[:, :, 0],
                                    in1=e3v[:, :, 1], op=AL.add)
            nc.vector.tensor_tensor(out=s3s[:], in0=s3s[:],
                                    in1=e3v[:, :, 2], op=AL.add)
            r3s = work.tile([P, C], f32, tag="r3s")
            nc.vector.reciprocal(out=r3s[:], in_=s3s[:])
            nc.vector.tensor_scalar_mul(out=r3s[:], in0=r3s[:],
                                        scalar1=lw_sb[:, b:b + 1])
            w_b = work.tile([P, C * S], bf16, tag="wb")
            wbv = w_b[:].rearrange("p (c s) -> p c s", s=S)
            for s in range(S):
                nc.vector.tensor_tensor(out=wbv[:, :, s], in0=e3v[:, :, s],
                                        in1=r3s[:], op=AL.mult)
            # hidden^T column: sum_n w_n * emb_n (PE over partitions, 12 blocks)
            hid_ps = pacc.tile([P, 1], f32, tag="acc")
            for j in range(C * S):
                nc.tensor.matmul(out=hid_ps[:],
                                 lhsT=emb_b[:, j * D:(j + 1) * D],
                                 rhs=w_b[:, j:j + 1],
                                 start=(j == 0), stop=(j == C * S - 1))
            nc.vector.tensor_copy(out=hidT[:, b:b + 1], in_=hid_ps[:])

        # ---- all-gather hidden: [8,128] local -> [64,128] global ----
        ht_ps = ptp.tile([BSH, P], f32, tag="tp")
        nc.tensor.transpose(out=ht_ps[:], in_=hidT[:], identity=ident_sb[:])
        hid8 = big.tile([BSH, P], f32)
        nc.scalar.copy(out=hid8[:], in_=ht_ps[:])
        hin = dram.tile([BSH, P], f32)
        hout = dram.tile([B, P], f32)
        nc.sync.dma_start(out=hin[:], in_=hid8[:])
        nc.gpsimd.collective_compute(
            "AllGather",
            mybir.AluOpType.bypass,
            ins=[hin[:].opt()],
            outs=[hout[:].opt()],
            replica_groups=[list(range(N_CORES))],
        )
        hid64 = big.tile([B, P], f32)
        nc.sync.dma_start(out=hid64[:], in_=hout[:])
        h64_ps = ptp.tile([P, B], f32, tag="tp")
        nc.tensor.transpose(out=h64_ps[:], in_=hid64[:],
                            identity=ident_sb[:B, :B])
        hidT64 = big.tile([P, B], bf16)
        nc.scalar.copy(out=hidT64[:], in_=h64_ps[:])

        # ---- logits shard + exp-sum stats ----
        y_all = big.tile([B, VSH], f32)
        acc = big.tile([B, 16], f32)
        chs = _chunks()
        for ci, (off, n) in enumerate(chs):
            lp = plog.tile([B, NCHUNK], f32, tag="log")
            nc.tensor.matmul(out=lp[:, :n], lhsT=hidT64[:],
                             rhs=w_sb[:, off:off + n], start=True, stop=True)
            nc.vector.tensor_tensor(out=y_all[:, off:off + n], in0=lp[:, :n],
                                    in1=b_bc[:, off:off + n], op=AL.add)
            esc = escp.tile([B, NCHUNK], f32, tag="esc")
            nc.scalar.activation(out=esc[:, :n], in_=y_all[:, off:off + n],
                                 func=AF.Exp, accum_out=acc[:, ci:ci + 1])
        sloc = big.tile([B, 1], f32)
        nc.vector.reduce_sum(out=sloc[:], in_=acc[:, :len(chs)], axis=AX.X)

        # ---- all-gather per-core exp-sums, combine, normalize ----
        sin = dram.tile([B, 1], f32)
        sout = dram.tile([N_CORES, B], f32)
        nc.sync.dma_start(out=sin[:], in_=sloc[:])
        nc.gpsimd.collective_compute(
            "AllGather",
            mybir.AluOpType.bypass,
            ins=[sin[:].opt()],
            outs=[sout[:].opt()],
            replica_groups=[list(range(N_CORES))],
        )
        s8 = big.tile([N_CORES, B], f32)
        nc.sync.dma_start(out=s8[:], in_=sout[:])
        st_ps = ptp.tile([B, 1], f32, tag="tp")
        nc.tensor.matmul(out=st_ps[:], lhsT=s8[:], rhs=ones8[:],
                         start=True, stop=True)
        logz = big.tile([B, 1], f32)
        nc.scalar.activation(out=logz[:], in_=st_ps[:], func=AF.Ln)
        nc.vector.tensor_scalar_sub(out=y_all[:], in0=y_all[:],
                                    scalar1=logz[:])
        nc.sync.dma_start(out=out, in_=y_all[:])

    nc.compile()
    return nc


def _wrap16(v):
    """dma_gather index layout: position i -> (i % 16, i // 16), replicated
    onto 128 partitions (8 Q7 cores x 16)."""
    w = v.reshape(-1, 16).T
    return np.ascontiguousarray(np.tile(w, (8, 1)))


def prepare_in_maps(inputs):
    import ml_dtypes

    bf16 = ml_dtypes.bfloat16
    inp = np.asarray(inputs["inputs"]).astype(np.int64)           # [64, 1536]
    lw = np.asarray(inputs["length_weights"]).astype(np.float32).reshape(B)
    mask = np.asarray(inputs["word_attn_mask"]).astype(bool)      # [64, 512]
    emb = np.asarray(inputs["embedding"]).astype(np.float32).copy()
    emb[0, :] = 0.0                                               # padding row
    w_attn = np.asarray(inputs["W_attn"]).astype(np.float32).reshape(D)
    # b_attn is softmax-invariant (constant shift before word softmax): ignored
    w_lin = np.asarray(inputs["W_lin"]).astype(np.float32)        # [50000, 128]
    b_lin = np.asarray(inputs["b_lin"]).astype(np.float32).reshape(OV)

    emb16 = emb.astype(bf16)                                      # one cast

    wt = np.ascontiguousarray(w_lin.T).astype(bf16)               # [128, 50000]
    w4 = np.tile((w_attn / 3.0), C)[None, :].astype(bf16)         # [1, 512]
    ident = np.eye(P, dtype=np.float32)

    # token order within a half: i = (b_loc*12 + c*3 + s)*128 + p
    # maps token (b = h*4+b_loc, l = c*128+p, sense s)
    idx6 = inp.reshape(N_CORES, NH, BH, C, P, S)       # (core,h,bl,c,p,s)
    pos = idx6.transpose(0, 1, 2, 3, 5, 4).reshape(N_CORES, NH, BH * C * S, P)
    pos = pos.transpose(0, 1, 3, 2)                    # (core, h, p, f)
    # flat order i = f*128 + p:
    flat = pos.transpose(0, 1, 3, 2).reshape(N_CORES, NH, NIDX_H)

    mb6 = np.where(mask, MASK_NEG, np.float32(0.0)).astype(
        np.float32).reshape(N_CORES, BSH, C, P)
    maskb_dev = np.ascontiguousarray(
        mb6.transpose(0, 3, 1, 2).reshape(N_CORES, P, BSH * C))
    lw_dev = lw.reshape(N_CORES, 1, BSH)

    in_maps = []
    for c in range(N_CORES):
        # per-core compact table: dedup the distinct rows this core touches
        used, inv = np.unique(flat[c], return_inverse=True)
        assert used.size <= TROWS
        tbl = np.zeros((TROWS, D), dtype=bf16)
        tbl[:used.size] = emb16[used]
        remap = inv.reshape(NH, NIDX_H).astype(np.int16)
        m = {
            "table": tbl,
            "maskb": maskb_dev[c],
            "w4": w4,
            "lw": np.ascontiguousarray(lw_dev[c]),
            "wlint": np.ascontiguousarray(wt[:, c * VSH:(c + 1) * VSH]),
            "blin": np.ascontiguousarray(b_lin[c * VSH:(c + 1) * VSH][None, :]),
            "ident": ident,
        }
        for h in range(NH):
            m[f"idx{h}"] = _wrap16(remap[h])
        in_maps.append(m)
    return in_maps


def _install_ntff_hook():
    """Provide antenv.axon_hooks (NTFF profiling glue) if the image lacks it.

    bass_utils hard-imports it on the trace=True path; this container's
    antenv package does not ship the module even though the axon .so
    supports profiling.  No-op if the real module exists or anything fails.
    """
    try:
        import importlib.util
        if "antenv.axon_hooks" in sys.modules:
            return
        try:
            if importlib.util.find_spec("antenv.axon_hooks") is not None:
                return
        except ModuleNotFoundError:
            pass
        import contextlib
        import ctypes
        import types

        so_path = "/opt/axon/libaxon_pjrt.so"
        if not os.path.exists(so_path):
            return
        lib = ctypes.CDLL(so_path)
        if not hasattr(lib, "axon_start_nrt_profile"):
            return
        lib.axon_start_nrt_profile.argtypes = [
            ctypes.POINTER(ctypes.c_int64), ctypes.c_size_t]
        lib.axon_start_nrt_profile.restype = ctypes.c_int64
        lib.axon_stop_nrt_profile.argtypes = [ctypes.c_char_p]
        lib.axon_stop_nrt_profile.restype = ctypes.c_int64

        @contextlib.contextmanager
        def _hook(output_dir, device_ids):
            import jax
            jax.devices()
            if device_ids:
                ids = (ctypes.c_int64 * len(device_ids))(*device_ids)
                rc = lib.axon_start_nrt_profile(ids, len(device_ids))
            else:
                rc = lib.axon_start_nrt_profile(None, 0)
            if rc != 0:
                raise RuntimeError(f"axon_start_nrt_profile rc={rc}")
            try:
                yield
            finally:
                n = lib.axon_stop_nrt_profile(str(output_dir).encode())
                print(f"profile: {n} file(s) written to {output_dir}",
                      file=sys.stderr)

        mod = types.ModuleType("antenv.axon_hooks")
        mod.get_axon_ntff_profile_hook = lambda: _hook
        mod.set_axon_ntff_profile_hook = lambda h: None
        sys.modules["antenv.axon_hooks"] = mod
        try:
            import antenv
            antenv.axon_hooks = mod
        except Exception:
            pass
    except Exception:
        pass


def kernel(**inputs):
    global LAST_EXEC_NS, LAST_RESULTS
    _install_ntff_hook()
    from concourse import bass_utils

    nc = build_nc()
    in_maps = prepare_in_maps(inputs)
    res = bass_utils.run_bass_kernel_spmd(
        nc, in_maps, core_ids=list(range(N_CORES)))
    LAST_EXEC_NS = res.exec_time_ns
    LAST_RESULTS = res
    return np.concatenate(
        [res.results[c]["out"] for c in range(N_CORES)], axis=1
    ).astype(np.float32)
